# revision 7
# baseline (speedup 1.0000x reference)
"""YOLOv3 detection-layer kernel for Trainium2 (Bass/Tile), 8-core data parallel.

Math (per image, input x [255, 5776] channel-major, f = a*85 + c):
  out_flat[hw, f] = g_f(x[f, hw])   where out_flat is [5776, 255] and the
  full output [17328, 85] is just out_flat reshaped (box = hw*3 + a).
So the kernel is: DMA load (channels on partitions) -> PE transpose-mode
(128x128 tiles, exact routing) into PSUM [hw, 255] -> fused sigmoid/exp +
grid/anchor affine -> contiguous DMA store.

Per anchor a (cols base = 85*a):
  sxy  = sigmoid(x[base+0:2])                 -> imxy = sxy*(1.05/76) + (g-0.025)/76
  half = exp(x[base+2:4]) * anchor_wh/(2*608)
  out[base+0:2] = imxy - half ; out[base+2:4] = imxy + half
  out[base+4:85] = sigmoid(x[base+4:85])
"""

import numpy as np

import concourse.bacc as bacc
import concourse.bass as bass
import concourse.mybir as mybir
import concourse.tile as tile
from concourse.bass_utils import run_bass_kernel_spmd
from concourse.masks import make_identity

F32 = mybir.dt.float32

B = 32            # batch
NCH = 255         # channels = 3 anchors * 85 attrs
H = W = 76
HW = H * W        # 5776
NCORES = 8
IPC = B // NCORES  # images per core
XY_SCALE = 1.05
KSC = XY_SCALE / W
ANCHOR_WH = [(10.0, 13.0), (16.0, 30.0), (33.0, 23.0)]

# Each group owns 4 PSUM banks and covers 512 (tail: 144) consecutive output
# rows. Within a group, PSUM partition p of bank t holds output row
# base + 4p + t, so each partition stores ONE contiguous 4080B DRAM chunk
# (4 adjacent 1020B rows) -> 128 descriptors per store instead of 512.
# (group_index, partitions) ; group 11 is the 144-row tail (36 partitions).
GROUPS = [(g, 128) for g in range(11)] + [(11, 36)]

SIG = mybir.ActivationFunctionType.Sigmoid
EXP = mybir.ActivationFunctionType.Exp

last_exec_time_ns = None
_cached = None


def _host_consts():
    # grid[p, s, a*2+c]: slot s = g*4+t covers output row hw = g*512 + 4p + t
    p = np.arange(128, dtype=np.int64)[:, None]
    s = np.arange(48, dtype=np.int64)[None, :]
    hw = (s // 4) * 512 + 4 * p + (s % 4)
    hw = np.minimum(hw, HW - 1)  # pad slots past the end; never read
    gx = (hw % W).astype(np.float64)
    gy = (hw // W).astype(np.float64)
    g = np.empty((128, 48, 6), dtype=np.float64)
    for a in range(3):
        g[:, :, 2 * a + 0] = (gx - 0.5 * (XY_SCALE - 1.0)) / W
        g[:, :, 2 * a + 1] = (gy - 0.5 * (XY_SCALE - 1.0)) / H
    grid = g.astype(np.float32).copy()
    anch = np.empty((6,), dtype=np.float64)
    for a in range(3):
        anch[2 * a + 0] = ANCHOR_WH[a][0] / (2.0 * 608.0)
        anch[2 * a + 1] = ANCHOR_WH[a][1] / (2.0 * 608.0)
    anch = np.broadcast_to(anch, (128, 4, 6)).astype(np.float32).copy()
    return grid, anch


def _build():
    nc = bacc.Bacc("TRN2", target_bir_lowering=False, debug=False, num_devices=NCORES)
    xt = nc.dram_tensor("x", [IPC, NCH, HW], F32, kind="ExternalInput").ap()
    gt = nc.dram_tensor("grid", [128, 48, 6], F32, kind="ExternalInput").ap()
    at = nc.dram_tensor("anch", [128, 4, 6], F32, kind="ExternalInput").ap()
    ot = nc.dram_tensor("out", [IPC, HW, NCH], F32, kind="ExternalOutput").ap()

    with tile.TileContext(nc) as tc:
        with (
            tc.tile_pool(name="consts", bufs=1) as consts,
            tc.tile_pool(name="xin", bufs=2) as xin,
            tc.tile_pool(name="psum", bufs=2, space="PSUM") as pp,
            tc.tile_pool(name="outp", bufs=3) as outp,
            tc.tile_pool(name="tmp", bufs=3) as tmpp,
        ):
            ident = consts.tile([128, 128], F32)
            make_identity(nc, ident)
            grid = consts.tile([128, 48, 6], F32)
            nc.sync.dma_start(grid, gt)
            anch = consts.tile([128, 4, 6], F32)
            nc.sync.dma_start(anch, at)

            for img in range(IPC):
                x0 = xin.tile([128, HW], F32, tag="x0")
                x1 = xin.tile([128, HW], F32, tag="x1")
                nc.sync.dma_start(x0, xt[img, 0:128, :])
                nc.sync.dma_start(x1[0:127], xt[img, 128:255, :])
                # [ch, hw] viewed as [ch, hw4, four]: column (m, t) is hw = 4m+t
                x0v = x0.rearrange("k (m four) -> k m four", four=4)
                x1v = x1[0:127].rearrange("k (m four) -> k m four", four=4)

                for g, P in GROUPS:
                    m0 = g * 128  # first hw4 column of this group
                    ps = pp.tile([128, 4, 512], F32, tag="ps")
                    for t in range(4):
                        nc.tensor.transpose(
                            ps[0:P, t, 0:128], x0v[:, m0 : m0 + P, t], ident
                        )
                        nc.tensor.transpose(
                            ps[0:P, t, 128:255],
                            x1v[:, m0 : m0 + P, t],
                            ident[0:127, 0:127],
                        )
                    o = outp.tile([128, 4, 255], F32, tag="o")
                    sxy = tmpp.tile([128, 4, 6], F32, tag="sxy")
                    ewh = tmpp.tile([128, 4, 6], F32, tag="ewh")
                    half = tmpp.tile([128, 4, 6], F32, tag="half")

                    v = ps[0:P, :, 0:255].rearrange("p t (a c) -> p t a c", a=3)
                    ov = o[0:P, :, :].rearrange("p t (a c) -> p t a c", a=3)
                    sv = sxy[0:P, :, :].rearrange("p t (a c) -> p t a c", a=3)
                    ev = ewh[0:P, :, :].rearrange("p t (a c) -> p t a c", a=3)
                    hv = half[0:P, :, :].rearrange("p t (a c) -> p t a c", a=3)
                    av = anch[0:P, :, :].rearrange("p t (a c) -> p t a c", a=3)
                    gv = grid[0:P, 4 * g : 4 * g + 4, :].rearrange(
                        "p t (a c) -> p t a c", a=3
                    )

                    nc.scalar.activation(ov[:, :, :, 4:85], v[:, :, :, 4:85], SIG)
                    nc.scalar.activation(sv, v[:, :, :, 0:2], SIG)
                    nc.scalar.activation(ev, v[:, :, :, 2:4], EXP)

                    nc.vector.tensor_mul(hv, ev, av)
                    nc.vector.tensor_scalar_mul(sxy[0:P], sxy[0:P], KSC)
                    nc.vector.tensor_add(sv, sv, gv)
                    nc.vector.tensor_sub(ov[:, :, :, 0:2], sv, hv)
                    nc.vector.tensor_add(ov[:, :, :, 2:4], sv, hv)

                    # rows g*512 + 4p + t ; per partition one 4080B chunk
                    dst = ot[img, g * 512 : g * 512 + 4 * P, :].rearrange(
                        "(p four) c -> p four c", four=4
                    )
                    nc.scalar.dma_start(dst, o[0:P, :, :])
    return nc


def kernel(x):
    global last_exec_time_ns, _cached
    x = np.ascontiguousarray(np.asarray(x, dtype=np.float32))
    assert x.shape == (B, NCH, H, W)
    if _cached is None:
        _cached = _build()
        _cached.finalize()  # Bacc: legalize sync waits + freeze
    nc = _cached
    grid, anch = _host_consts()
    xr = x.reshape(B, NCH, HW)
    in_maps = [
        {"x": np.ascontiguousarray(xr[c * IPC : (c + 1) * IPC]), "grid": grid, "anch": anch}
        for c in range(NCORES)
    ]
    res = run_bass_kernel_spmd(nc, in_maps, core_ids=list(range(NCORES)))
    last_exec_time_ns = res.exec_time_ns
    out = np.concatenate(
        [r["out"].reshape(IPC, HW * 3, 85) for r in res.results], axis=0
    )
    return out


# revision 9
# speedup vs baseline: 1.0496x; 1.0496x over previous
"""YOLOv3 detection-layer kernel for Trainium2 (Bass/Tile), 8-core data parallel.

Math (per image, input x [255, 5776] channel-major, f = a*85 + c):
  out_flat[hw, f] = g_f(x[f, hw])   where out_flat is [5776, 255] and the
  full output [17328, 85] is just out_flat reshaped (box = hw*3 + a).
So the kernel is: DMA load (channels on partitions) -> PE transpose-mode
(128x128 tiles, exact routing) into PSUM [hw, 255] -> fused sigmoid/exp +
grid/anchor affine -> contiguous DMA store.

Per anchor a (cols base = 85*a):
  sxy  = sigmoid(x[base+0:2])                 -> imxy = sxy*(1.05/76) + (g-0.025)/76
  half = exp(x[base+2:4]) * anchor_wh/(2*608)
  out[base+0:2] = imxy - half ; out[base+2:4] = imxy + half
  out[base+4:85] = sigmoid(x[base+4:85])
"""

import numpy as np

import concourse.bacc as bacc
import concourse.bass as bass
import concourse.mybir as mybir
import concourse.tile as tile
from concourse.bass_utils import run_bass_kernel_spmd
from concourse.masks import make_identity

F32 = mybir.dt.float32

B = 32            # batch
NCH = 255         # channels = 3 anchors * 85 attrs
H = W = 76
HW = H * W        # 5776
NCORES = 8
IPC = B // NCORES  # images per core
XY_SCALE = 1.05
KSC = XY_SCALE / W
ANCHOR_WH = [(10.0, 13.0), (16.0, 30.0), (33.0, 23.0)]

# Each group owns 4 PSUM banks and covers 512 (tail: 144) consecutive output
# rows. Within a group, PSUM partition p of bank t holds output row
# base + 4p + t, so each partition stores ONE contiguous 4080B DRAM chunk
# (4 adjacent 1020B rows) -> 128 descriptors per store instead of 512.
# (group_index, partitions) ; group 11 is the 144-row tail (36 partitions).
GROUPS = [(g, 128) for g in range(11)] + [(11, 36)]

SIG = mybir.ActivationFunctionType.Sigmoid
EXP = mybir.ActivationFunctionType.Exp

last_exec_time_ns = None
_cached = None


def _host_consts():
    # grid[p, s, a*2+c]: slot s = g*4+t covers output row hw = g*512 + 4p + t
    p = np.arange(128, dtype=np.int64)[:, None]
    s = np.arange(48, dtype=np.int64)[None, :]
    hw = (s // 4) * 512 + 4 * p + (s % 4)
    hw = np.minimum(hw, HW - 1)  # pad slots past the end; never read
    gx = (hw % W).astype(np.float64)
    gy = (hw // W).astype(np.float64)
    g = np.empty((128, 48, 6), dtype=np.float64)
    for a in range(3):
        g[:, :, 2 * a + 0] = (gx - 0.5 * (XY_SCALE - 1.0)) / W
        g[:, :, 2 * a + 1] = (gy - 0.5 * (XY_SCALE - 1.0)) / H
    grid = g.astype(np.float32).copy()
    anch = np.empty((6,), dtype=np.float64)
    for a in range(3):
        anch[2 * a + 0] = ANCHOR_WH[a][0] / (2.0 * 608.0)
        anch[2 * a + 1] = ANCHOR_WH[a][1] / (2.0 * 608.0)
    anch = np.broadcast_to(anch, (128, 4, 6)).astype(np.float32).copy()
    return grid, anch


def _build():
    nc = bacc.Bacc("TRN2", target_bir_lowering=False, debug=False, num_devices=NCORES)
    xt = nc.dram_tensor("x", [IPC, NCH, HW], F32, kind="ExternalInput").ap()
    gt = nc.dram_tensor("grid", [128, 48, 6], F32, kind="ExternalInput").ap()
    at = nc.dram_tensor("anch", [128, 4, 6], F32, kind="ExternalInput").ap()
    ot = nc.dram_tensor("out", [IPC, HW, NCH], F32, kind="ExternalOutput").ap()

    with tile.TileContext(nc) as tc:
        with (
            tc.tile_pool(name="consts", bufs=1) as consts,
            tc.tile_pool(name="xin", bufs=2) as xin,
            tc.tile_pool(name="psum", bufs=2, space="PSUM") as pp,
            tc.tile_pool(name="outp", bufs=3) as outp,
            tc.tile_pool(name="tmp", bufs=3) as tmpp,
        ):
            ident = consts.tile([128, 128], F32)
            make_identity(nc, ident)
            grid = consts.tile([128, 48, 6], F32)
            nc.sync.dma_start(grid, gt)
            anch = consts.tile([128, 4, 6], F32)
            nc.sync.dma_start(anch, at)

            for img in range(IPC):
                x0 = xin.tile([128, HW], F32, tag="x0")
                x1 = xin.tile([128, HW], F32, tag="x1")
                nc.sync.dma_start(x0, xt[img, 0:128, :])
                nc.sync.dma_start(x1[0:127], xt[img, 128:255, :])
                # [ch, hw] viewed as [ch, hw4, four]: column (m, t) is hw = 4m+t
                x0v = x0.rearrange("k (m four) -> k m four", four=4)
                x1v = x1[0:127].rearrange("k (m four) -> k m four", four=4)

                for g, P in GROUPS:
                    m0 = g * 128  # first hw4 column of this group
                    ps = pp.tile([128, 4, 512], F32, tag="ps")
                    for t in range(4):
                        nc.tensor.transpose(
                            ps[0:P, t, 0:128], x0v[:, m0 : m0 + P, t], ident
                        )
                        nc.tensor.transpose(
                            ps[0:P, t, 128:255],
                            x1v[:, m0 : m0 + P, t],
                            ident[0:127, 0:127],
                        )
                    o = outp.tile([128, 4, 255], F32, tag="o")
                    sxy = tmpp.tile([128, 4, 6], F32, tag="sxy")
                    ewh = tmpp.tile([128, 4, 6], F32, tag="ewh")
                    neg = tmpp.tile([128, 4, 6], F32, tag="neg")
                    half = tmpp.tile([128, 4, 6], F32, tag="half")

                    v = ps[0:P, :, 0:255].rearrange("p t (a c) -> p t a c", a=3)
                    ov = o[0:P, :, :].rearrange("p t (a c) -> p t a c", a=3)
                    sv = sxy[0:P, :, :].rearrange("p t (a c) -> p t a c", a=3)
                    ev = ewh[0:P, :, :].rearrange("p t (a c) -> p t a c", a=3)
                    nv = neg[0:P, :, :].rearrange("p t (a c) -> p t a c", a=3)
                    hv = half[0:P, :, :].rearrange("p t (a c) -> p t a c", a=3)
                    av = anch[0:P, :, :].rearrange("p t (a c) -> p t a c", a=3)
                    gv = grid[0:P, 4 * g : 4 * g + 4, :].rearrange(
                        "p t (a c) -> p t a c", a=3
                    )

                    # exp(x) = sigmoid(x)/sigmoid(-x): stays in the sigmoid
                    # act table (avoids a LoadActFuncSet per group, ~640ns each)
                    nc.scalar.activation(ov[:, :, :, 4:85], v[:, :, :, 4:85], SIG)
                    nc.scalar.activation(sv, v[:, :, :, 0:2], SIG)
                    nc.scalar.activation(ev, v[:, :, :, 2:4], SIG)
                    nc.scalar.activation(nv, v[:, :, :, 2:4], SIG, scale=-1.0)

                    nc.vector.reciprocal(neg[0:P], neg[0:P])
                    nc.vector.tensor_mul(ev, ev, nv)
                    nc.vector.tensor_mul(hv, ev, av)
                    nc.vector.tensor_scalar_mul(sxy[0:P], sxy[0:P], KSC)
                    nc.vector.tensor_add(sv, sv, gv)
                    nc.vector.tensor_sub(ov[:, :, :, 0:2], sv, hv)
                    nc.vector.tensor_add(ov[:, :, :, 2:4], sv, hv)

                    # rows g*512 + 4p + t ; per partition one 4080B chunk
                    dst = ot[img, g * 512 : g * 512 + 4 * P, :].rearrange(
                        "(p four) c -> p four c", four=4
                    )
                    nc.scalar.dma_start(dst, o[0:P, :, :])
    return nc


def kernel(x):
    global last_exec_time_ns, _cached
    x = np.ascontiguousarray(np.asarray(x, dtype=np.float32))
    assert x.shape == (B, NCH, H, W)
    if _cached is None:
        _cached = _build()
        _cached.finalize()  # Bacc: legalize sync waits + freeze
    nc = _cached
    grid, anch = _host_consts()
    xr = x.reshape(B, NCH, HW)
    in_maps = [
        {"x": np.ascontiguousarray(xr[c * IPC : (c + 1) * IPC]), "grid": grid, "anch": anch}
        for c in range(NCORES)
    ]
    res = run_bass_kernel_spmd(nc, in_maps, core_ids=list(range(NCORES)))
    last_exec_time_ns = res.exec_time_ns
    out = np.concatenate(
        [r["out"].reshape(IPC, HW * 3, 85) for r in res.results], axis=0
    )
    return out
